# revision 17
# baseline (speedup 1.0000x reference)
"""Trainium2 Bass kernel for GQA attention (B=2, S=2048, D=2048, H=32, KVH=8).

Sharding: batch data-parallel across 2 groups of 4 cores; within a group,
4-way tensor parallel over heads (8 q heads + their 2 kv heads per core).
Device-side per-chunk ReduceScatter(add) over each 4-core group after the wo
matmul; the host concatenates the token slices.

The device program is identical on all 8 cores (SPMD); all per-core
variation (batch slice, head slice) is carried by the input data.

v2 structure: a software pipeline over 512-token chunks —
   proj(c) -> rope(c) -> kTrep/v(c) -> attention(c) -> wo(c) -> RS(c)
so the tensor engine stays dense (HAM stays warm) and the collective
overlaps compute. Projections and the wo matmul run in fp32r; the
attention path (q/k/v/probs/cos/mask) runs in bf16 (fp32 PSUM accum).

Layout notes:
 - Host passes x pre-transposed (xT, chunk-major); every matmul consumes xT
   directly (no on-device transposes of activations).
 - wq/wk columns are permuted on host into an "even dims block / odd dims
   block" (A/B) layout so RoPE is full-partition DVE work; wq carries the
   1/sqrt(HD) scale (exact power of two).
 - Scores are computed transposed (scoresT[sk, sq]) so probsT feeds the AV
   matmul directly with no transposes in the attention path.
 - Causal mask: one extra accumulating matmul per diagonal tile,
   LEones[k,p]=[k<=p] x shifted -1e9 diagonal, adds -1e9 to masked entries.
 - Softmax denominators ride along as a ones column in v (M=65 AV matmul);
   normalization multiplies by the partition-broadcast reciprocal.
"""

import os
import sys
import functools

import numpy as np

if "/opt/trn_rl_repo" not in sys.path:
    sys.path.insert(0, "/opt/trn_rl_repo")

B, S, D = 2, 2048, 2048
H, KVH = 32, 8
HD = D // H            # 64
N_CORES = 8
GROUP = 4              # cores per batch group (tensor parallel width)
HPC = 8                # query heads per core
KVPC = 2               # kv heads per core
SQC = 512              # sq chunk (psum bank width in fp32)
PT = 128               # partition tile
KT = D // PT           # 16 contraction tiles
NT = S // PT           # 16 token tiles
NCHUNK = S // SQC      # 4
TPC = SQC // PT        # tok tiles per chunk (4)
MASK_W = 896           # width of the shifted-diagonal mask table
NEG = -1e9


def _build_program(mm_dtype_name="float32r"):
    import concourse.bass as bass
    import concourse.bacc as bacc
    import concourse.mybir as mybir
    import concourse.tile as tile
    import ml_dtypes
    from contextlib import ExitStack

    f32 = mybir.dt.float32
    bf16 = mybir.dt.bfloat16
    mmdt = getattr(mybir.dt, mm_dtype_name)

    def mc(ap):  # bitcast for fp32r matmul operands/producers
        return ap.bitcast(mmdt) if mmdt != f32 else ap

    nc = bacc.Bacc("TRN2", target_bir_lowering=False, debug=False,
                   num_devices=N_CORES)

    # ---- dram parameters -------------------------------------------------
    xT_d = nc.dram_tensor("xt", [NCHUNK, D, SQC], f32, kind="ExternalInput")
    wq_d = nc.dram_tensor("wq", [D, HPC * HD], f32, kind="ExternalInput")
    wk_d = nc.dram_tensor("wk", [D, KVPC * HD], f32, kind="ExternalInput")
    wv_d = nc.dram_tensor("wv", [D, KVPC * HD], f32, kind="ExternalInput")
    wo_d = nc.dram_tensor("wo", [HPC * HD, D], f32, kind="ExternalInput")
    cos_d = nc.dram_tensor("cosr", [PT, S], bf16, kind="ExternalInput")
    sin_d = nc.dram_tensor("sinr", [PT, S], bf16, kind="ExternalInput")
    y_out = nc.dram_tensor("y", [S // GROUP, D], f32, kind="ExternalOutput")

    y_part = nc.dram_tensor("y_part", [S, D], f32)
    y_rs = nc.dram_tensor("y_rs", [S // GROUP, D], f32)

    # ---- inline constants ------------------------------------------------
    leones = np.zeros((PT, PT), np.float32)      # leones[k, p] = 1 if k <= p
    for k in range(PT):
        leones[k, k:] = 1.0
    dmaster = np.zeros((PT, MASK_W), np.float32)
    # col m: -1e9 one-hot at k = m-383 for m in [384, 510]; row 0 for m<384
    dmaster[0, :384] = NEG
    for m in range(384, 511):
        dmaster[m - 383, m] = NEG
    ident = np.eye(PT, dtype=np.float32)
    ones1 = np.ones((PT, 1), ml_dtypes.bfloat16)

    le_d = nc.inline_tensor(leones.astype(ml_dtypes.bfloat16), "leones")
    dm_d = nc.inline_tensor(dmaster.astype(ml_dtypes.bfloat16), "dmaster")
    id_d = nc.inline_tensor(ident, "ident")
    on_d = nc.inline_tensor(ones1, "ones1")

    Exp = mybir.ActivationFunctionType.Exp
    groups = [[0, 1, 2, 3], [4, 5, 6, 7]]

    with tile.TileContext(nc) as tc, ExitStack() as ctx:
        keep = ctx.enter_context(tc.tile_pool(name="keep", bufs=1))
        kTrep = keep.tile([PT, 4, S], bf16)    # Akv0 Akv1 Bkv0 Bkv1 (x4 rows)
        v_sb = keep.tile([PT, KVPC, NT, HD + 1], bf16)   # col 64 = ones
        cos_sb = keep.tile([PT, S], bf16)
        sin_sb = keep.tile([PT, S], bf16)
        le_sb = keep.tile([PT, PT], bf16)
        dm_sb = keep.tile([PT, MASK_W], bf16)
        id_sb = keep.tile([PT, PT], f32)
        wq_sb = keep.tile([PT, KT, HPC * HD], f32)
        wk_sb = keep.tile([PT, KT, KVPC * HD], f32)
        wv_sb = keep.tile([PT, KT, KVPC * HD], f32)
        wo_sb = keep.tile([PT, 4, D], f32)

        nc.sync.dma_start(out=le_sb[:], in_=le_d[:])
        nc.sync.dma_start(out=dm_sb[:], in_=dm_d[:])
        nc.sync.dma_start(out=id_sb[:], in_=id_d[:])
        nc.sync.dma_start(out=cos_sb[:], in_=cos_d[:])
        nc.sync.dma_start(out=sin_sb[:], in_=sin_d[:])
        nc.sync.dma_start(out=mc(wq_sb[:]),
                          in_=mc(wq_d.ap().rearrange("(k p) n -> p k n", p=PT)))
        nc.sync.dma_start(out=mc(wk_sb[:]),
                          in_=mc(wk_d.ap().rearrange("(k p) n -> p k n", p=PT)))
        nc.sync.dma_start(out=mc(wv_sb[:]),
                          in_=mc(wv_d.ap().rearrange("(k p) n -> p k n", p=PT)))
        nc.sync.dma_start(out=mc(wo_sb[:]),
                          in_=mc(wo_d.ap().rearrange("(k p) n -> p k n", p=PT)))
        # ones column of v (every (kv, t) slot)
        ones_src = bass.AP(tensor=on_d.ap().tensor, offset=0,
                           ap=[[1, PT], [0, KVPC * NT], [1, 1]])
        vcol = v_sb[:, :, :, HD:HD + 1]
        ones_dst = bass.AP(tensor=vcol.tensor, offset=vcol.offset,
                           ap=[list(vcol.ap[0]), [HD + 1, KVPC * NT], [1, 1]])
        nc.sync.dma_start(out=ones_dst, in_=ones_src)

        xpool = ctx.enter_context(tc.tile_pool(name="xp", bufs=2))
        qpool = ctx.enter_context(tc.tile_pool(name="qp", bufs=2))
        kpool = ctx.enter_context(tc.tile_pool(name="kp", bufs=2))
        vtp = ctx.enter_context(tc.tile_pool(name="vtp", bufs=2))
        otp = ctx.enter_context(tc.tile_pool(name="otp", bufs=2))
        rtmp = ctx.enter_context(tc.tile_pool(name="rtmp", bufs=1))
        probs = ctx.enter_context(tc.tile_pool(name="probs", bufs=4))
        bcp = ctx.enter_context(tc.tile_pool(name="bcp", bufs=1))
        rcp = ctx.enter_context(tc.tile_pool(name="rcp", bufs=1))
        osg = ctx.enter_context(tc.tile_pool(name="osg", bufs=1))
        ysb = ctx.enter_context(tc.tile_pool(name="ysb", bufs=2))
        mw = ctx.enter_context(tc.tile_pool(name="mw", bufs=2, space="PSUM"))
        sps = ctx.enter_context(tc.tile_pool(name="sps", bufs=2, space="PSUM"))
        aps = ctx.enter_context(tc.tile_pool(name="aps", bufs=4, space="PSUM"))

        def rope_pair(a, b, cs, sn, nm):
            """a' = a*cos - b*sin ; b' = a*sin + b*cos (bf16, in place)."""
            t1 = rtmp.tile(a.shape, bf16, tag="t1", name=f"t1{nm}")
            t2 = rtmp.tile(a.shape, bf16, tag="t2", name=f"t2{nm}")
            t3 = rtmp.tile(a.shape, bf16, tag="t3", name=f"t3{nm}")
            nc.vector.tensor_mul(t1[:], a, cs)
            nc.vector.tensor_mul(t2[:], a, sn)
            nc.vector.tensor_mul(t3[:], b, sn)
            nc.vector.tensor_sub(a, t1[:], t3[:])
            t4 = rtmp.tile(a.shape, bf16, tag="t3", name=f"t4{nm}")
            nc.vector.tensor_mul(t4[:], b, cs)
            nc.vector.tensor_add(b, t2[:], t4[:])

        for c in range(NCHUNK):
            csl = slice(c * SQC, (c + 1) * SQC)

            # ---- proj(c): qT chunk (bf16), kT chunk (bf16), vT chunk ----
            halves = []
            for hf in range(2):
                xt = xpool.tile([PT, KT // 2, SQC], f32, tag="xt",
                                name=f"xt{c}_{hf}")
                nc.sync.dma_start(
                    out=mc(xt[:]),
                    in_=mc(xT_d[c].rearrange("(k p) n -> p k n", p=PT)
                           [:, hf * (KT // 2):(hf + 1) * (KT // 2), :]))
                halves.append(xt)

            qc = qpool.tile([PT, 4, SQC], bf16, tag="qc", name=f"qc{c}")
            kc = kpool.tile([PT, SQC], bf16, tag="kc", name=f"kc{c}")
            vtc = vtp.tile([PT, SQC], f32, tag="vtc", name=f"vtc{c}")
            for mt in range(4):
                ps = mw.tile([PT, SQC], f32, tag="ps", name=f"qps{c}_{mt}")
                for k in range(KT):
                    nc.tensor.matmul(
                        ps[:], mc(wq_sb[:, k, mt * PT:(mt + 1) * PT]),
                        mc(halves[k // 8][:, k % 8, :]),
                        start=(k == 0), stop=(k == KT - 1))
                nc.scalar.copy(qc[:, mt, :], ps[:])
            for dst, wsb, nm in ((kc, wk_sb, "k"), (vtc, wv_sb, "v")):
                ps = mw.tile([PT, SQC], f32, tag="ps", name=f"ps{nm}{c}")
                for k in range(KT):
                    nc.tensor.matmul(
                        ps[:], mc(wsb[:, k, :]),
                        mc(halves[k // 8][:, k % 8, :]),
                        start=(k == 0), stop=(k == KT - 1))
                if dst is vtc:
                    nc.scalar.copy(mc(dst[:]), ps[:])
                else:
                    nc.scalar.copy(dst[:], ps[:])

            # ---- rope(c) ------------------------------------------------
            for j in range(2):
                rope_pair(qc[:, j, :], qc[:, 2 + j, :],
                          cos_sb[:, csl], sin_sb[:, csl], f"q{c}_{j}")
            # k pair: rows 0:64 / 64:128 — stage B rows to base 0 via DMA
            bst = rtmp.tile([64, SQC], bf16, tag="t1", name=f"bst{c}")
            nc.sync.dma_start(out=bst[:], in_=kc[64:128, :])
            kt1 = rtmp.tile([64, SQC], bf16, tag="t2", name=f"kt1{c}")
            kt2 = rtmp.tile([64, SQC], bf16, tag="t3", name=f"kt2{c}")
            kt3 = rtmp.tile([64, SQC], bf16, tag="t1b", name=f"kt3{c}")
            kt4 = rtmp.tile([64, SQC], bf16, tag="t2b", name=f"kt4{c}")
            nc.vector.tensor_mul(kt1[:], kc[0:64, :], cos_sb[0:64, csl])
            nc.vector.tensor_mul(kt2[:], kc[0:64, :], sin_sb[0:64, csl])
            nc.vector.tensor_mul(kt3[:], bst[:], sin_sb[0:64, csl])
            nc.vector.tensor_mul(kt4[:], bst[:], cos_sb[0:64, csl])
            nc.vector.tensor_sub(kc[0:64, :], kt1[:], kt3[:])
            kbr = rtmp.tile([64, SQC], bf16, tag="t3b", name=f"kbr{c}")
            nc.vector.tensor_add(kbr[:], kt2[:], kt4[:])
            nc.sync.dma_start(out=kc[64:128, :], in_=kbr[:])

            # ---- kTrep(c): each 32-row group replicated x4 --------------
            for r in range(4):
                for slot in range(4):
                    nc.sync.dma_start(
                        out=kTrep[slot * 32:(slot + 1) * 32, r, csl],
                        in_=kc[r * 32:(r + 1) * 32, :])

            # ---- v(c): transpose vT chunk into v_sb ---------------------
            for tl in range(TPC):
                t = c * TPC + tl
                tp = mw.tile([PT, SQC], f32, tag="ps", name=f"tp{c}_{tl}")
                nc.tensor.transpose(tp[:, 0:PT],
                                    vtc[:, tl * PT:(tl + 1) * PT],
                                    id_sb[:])
                nc.vector.tensor_copy(v_sb[:, 0, t, 0:HD], tp[:, 0:HD])
                nc.vector.tensor_copy(v_sb[:, 1, t, 0:HD], tp[:, HD:2 * HD])

            # ---- attention(c) -------------------------------------------
            outc = otp.tile([PT, 4, SQC], f32, tag="outc", name=f"outc{c}")
            ntk = 4 * c + 4
            for g in range(2):
                av = [aps.tile([PT, SQC], f32, tag="av",
                               name=f"av{c}_{g}_{i}") for i in range(4)]
                for t in range(ntk):
                    ksl = slice(t * PT, (t + 1) * PT)
                    sc = [sps.tile([PT, SQC], f32, tag="sc",
                                   name=f"sc{c}_{g}_{t}_{i}")
                          for i in range(4)]
                    diag = t >= 4 * c
                    for m in range(4):
                        msl = slice(m * 32, (m + 1) * 32)
                        nc.tensor.matmul(
                            sc[m][:], kTrep[msl, g, ksl], qc[msl, g, :],
                            start=True, stop=False,
                            tile_position=(m * 32, 0))
                        nc.tensor.matmul(
                            sc[m][:], kTrep[msl, 2 + g, ksl],
                            qc[msl, 2 + g, :],
                            start=False, stop=not diag,
                            tile_position=(m * 32, 0))
                        if diag:
                            r = t - 4 * c
                            nc.tensor.matmul(
                                sc[m][:], le_sb[:],
                                dm_sb[:, 384 - 128 * r:MASK_W - 128 * r],
                                start=False, stop=True)
                    for m in range(4):
                        pb = probs.tile([PT, SQC], bf16, tag="pb",
                                        name=f"pb{c}_{g}_{t}_{m}")
                        nc.scalar.activation(pb[:], sc[m][:], Exp)
                        nc.tensor.matmul(
                            av[m][0:HD + 1, :], v_sb[:, g, t, :], pb[:],
                            start=(t == 0), stop=(t == ntk - 1))
                for m in range(4):
                    qh = g * 4 + m
                    rc = rcp.tile([1, SQC], f32, tag="rc",
                                  name=f"rc{c}_{g}_{m}")
                    nc.vector.reciprocal(rc[:], av[m][HD:HD + 1, :])
                    bc = bcp.tile([64, SQC], f32, tag="bc",
                                  name=f"bc{c}_{g}_{m}")
                    nc.gpsimd.partition_broadcast(bc[:], rc[:])
                    dst = outc[(qh % 2) * HD:(qh % 2 + 1) * HD, qh // 2, :]
                    if qh % 2 == 0:
                        nc.vector.tensor_mul(mc(dst), av[m][0:HD, :], bc[:])
                    else:
                        st = osg.tile([64, SQC], f32, tag="st",
                                      name=f"st{c}_{g}_{m}")
                        nc.vector.tensor_mul(st[:], av[m][0:HD, :], bc[:])
                        nc.sync.dma_start(out=mc(dst), in_=mc(st[:]))

            # ---- wo(c) --------------------------------------------------
            for tl in range(TPC):
                tt = c * TPC + tl
                yt = ysb.tile([PT, D], f32, tag="yt", name=f"yt{c}_{tl}")
                for nk in range(4):
                    yp = mw.tile([PT, SQC], f32, tag="ps",
                                 name=f"yp{c}_{tl}_{nk}")
                    for k4 in range(4):
                        nc.tensor.matmul(
                            yp[:], mc(outc[:, k4, tl * PT:(tl + 1) * PT]),
                            mc(wo_sb[:, k4, nk * SQC:(nk + 1) * SQC]),
                            start=(k4 == 0), stop=(k4 == 3))
                    nc.vector.tensor_copy(yt[:, nk * SQC:(nk + 1) * SQC],
                                          yp[:])
                nc.sync.dma_start(out=y_part[tt * PT:(tt + 1) * PT, :],
                                  in_=yt[:])

            # ---- RS(c): reduce-scatter this chunk's rows ----------------
            nc.gpsimd.collective_compute(
                "ReduceScatter", mybir.AluOpType.add, replica_groups=groups,
                ins=[y_part.ap()[csl, :]],
                outs=[y_rs.ap()[c * PT:(c + 1) * PT, :]])
            nc.sync.dma_start(out=y_out.ap()[c * PT:(c + 1) * PT, :],
                              in_=y_rs.ap()[c * PT:(c + 1) * PT, :])

    nc.compile()
    return nc


@functools.lru_cache(maxsize=2)
def _get_program(mm_dtype_name="float32r"):
    return _build_program(mm_dtype_name)


def _host_inputs(x, wq, wk, wv, wo, cos, sin):
    """Build the 8 per-core input maps."""
    import ml_dtypes

    perm_q = np.empty(HPC * HD, np.int64)
    for rho in range(HPC * HD):
        blk, rem = divmod(rho, HPC * HD // 2)
        h, i = divmod(rem, 32)
        perm_q[rho] = h * HD + 2 * i + blk
    perm_k = np.empty(KVPC * HD, np.int64)
    for rho in range(KVPC * HD):
        blk, rem = divmod(rho, KVPC * HD // 2)
        kv, i = divmod(rem, 32)
        perm_k[rho] = kv * HD + 2 * i + blk

    reps = np.tile(np.arange(32), 4)
    cosr = np.ascontiguousarray(cos.T[reps]).astype(ml_dtypes.bfloat16)
    sinr = np.ascontiguousarray(sin.T[reps]).astype(ml_dtypes.bfloat16)

    xts = []
    for b in range(B):
        xt = x[b].T.reshape(D, NCHUNK, SQC)       # [D, 4, 512]
        xts.append(np.ascontiguousarray(xt.transpose(1, 0, 2)))

    scale = np.float32(1.0 / np.sqrt(HD))
    in_maps = []
    for core in range(N_CORES):
        b, hg = divmod(core, GROUP)
        qcols = slice(hg * HPC * HD, (hg + 1) * HPC * HD)
        kcols = slice(hg * KVPC * HD, (hg + 1) * KVPC * HD)
        wq_c = (wq[:, qcols] * scale)[:, perm_q]
        wk_c = wk[:, kcols][:, perm_k]
        wv_c = np.ascontiguousarray(wv[:, kcols])
        wo_c = np.ascontiguousarray(wo[qcols, :])
        in_maps.append({
            "xt": xts[b],
            "wq": np.ascontiguousarray(wq_c),
            "wk": np.ascontiguousarray(wk_c),
            "wv": wv_c,
            "wo": wo_c,
            "cosr": cosr,
            "sinr": sinr,
        })
    return in_maps


def _assemble(results):
    """results[core]["y"] rows are [chunk(4) x 128] token blocks."""
    out = np.empty((B, S, D), np.float32)
    for b in range(B):
        for r in range(GROUP):
            y = results[b * GROUP + r]["y"]
            for c in range(NCHUNK):
                rows = slice(c * SQC + r * PT, c * SQC + (r + 1) * PT)
                out[b, rows, :] = y[c * PT:(c + 1) * PT, :]
    return out


def _is_causal(mask):
    if mask.shape != (S, S):
        return False
    expect = np.where(np.tril(np.ones((S, S), bool)), np.float32(0.0),
                      np.float32(NEG))
    return np.array_equal(mask, expect)


def _numpy_fallback(x, wq, wk, wv, wo, cos, sin, mask):
    """Exact reference math on host (only used if mask isn't causal)."""
    xq = (x @ wq).reshape(B, S, H, HD)
    xk = (x @ wk).reshape(B, S, KVH, HD)
    xv = (x @ wv).reshape(B, S, KVH, HD)

    def rope(t):
        tr = t.reshape(*t.shape[:-1], HD // 2, 2)
        a, b = tr[..., 0], tr[..., 1]
        c = cos[None, :, None, :]
        s_ = sin[None, :, None, :]
        out = np.stack([a * c - b * s_, a * s_ + b * c], axis=-1)
        return out.reshape(t.shape)

    xq, xk = rope(xq), rope(xk)
    xk = np.repeat(xk, H // KVH, axis=2)
    xv = np.repeat(xv, H // KVH, axis=2)
    q = xq.transpose(0, 2, 1, 3)
    k = xk.transpose(0, 2, 1, 3)
    v = xv.transpose(0, 2, 1, 3)
    sc = np.einsum("bhqd,bhkd->bhqk", q, k) / np.sqrt(np.float32(HD))
    sc = sc + mask[None, None]
    sc = sc - sc.max(-1, keepdims=True)
    p = np.exp(sc)
    p /= p.sum(-1, keepdims=True)
    out = np.einsum("bhqk,bhkd->bhqd", p, v)
    out = out.transpose(0, 2, 1, 3).reshape(B, S, H * HD)
    return (out @ wo).astype(np.float32)


def _ensure_ntff_hook():
    """Provide antenv.axon_hooks (missing on this image) so trace=True works."""
    try:
        from antenv.axon_hooks import get_axon_ntff_profile_hook  # noqa: F401
        return True
    except ImportError:
        pass
    try:
        import types
        import antenv
        from trn_agent_boot.trn_boot import _ntff_profile_via_ctypes

        mod = types.ModuleType("antenv.axon_hooks")
        _state = {"hook": None}
        mod.set_axon_ntff_profile_hook = \
            lambda h: _state.__setitem__("hook", h)
        mod.get_axon_ntff_profile_hook = lambda: _state["hook"]
        sys.modules["antenv.axon_hooks"] = mod
        antenv.axon_hooks = mod
        mod.set_axon_ntff_profile_hook(
            _ntff_profile_via_ctypes("/opt/axon/libaxon_pjrt.so"))
        return mod.get_axon_ntff_profile_hook() is not None
    except Exception:
        return False


def kernel(x, wq, wk, wv, wo, cos, sin, mask):
    x = np.asarray(x, np.float32)
    wq = np.asarray(wq, np.float32)
    wk = np.asarray(wk, np.float32)
    wv = np.asarray(wv, np.float32)
    wo = np.asarray(wo, np.float32)
    cos = np.asarray(cos, np.float32)
    sin = np.asarray(sin, np.float32)
    mask = np.asarray(mask, np.float32)

    if not _is_causal(mask):
        return _numpy_fallback(x, wq, wk, wv, wo, cos, sin, mask)

    from concourse.bass_utils import run_bass_kernel_spmd

    nc = _get_program(os.environ.get("ATTN_MM_DTYPE", "float32r"))
    in_maps = _host_inputs(x, wq, wk, wv, wo, cos, sin)
    trace = bool(int(os.environ.get("ATTN_TRACE", "0")))
    if trace and not _ensure_ntff_hook():
        trace = False
    res = run_bass_kernel_spmd(nc, in_maps, core_ids=list(range(N_CORES)),
                               trace=trace)
    if trace:
        kernel.last_exec_time_ns = res.exec_time_ns
        kernel.last_results = res
    return _assemble(res.results)
